# revision 9
# baseline (speedup 1.0000x reference)
"""Trainium2 Bass kernel for nn_Attention (b=8, n=1024, dim=768, heads=12).

Sharding: data-parallel over batch — 8 batch elements -> 8 NeuronCores.
Each core runs full attention for one [1024, 768] slice; weights replicated.

v2b: software-pipelined schedule.
- Head-at-a-time attention; attnV lags the ST/exp pipeline (lag 4 for head 0
  to ride out the wv DMA arrival, lag 1 after) so the exp (ACT engine)
  overlaps PE work instead of stalling it.
- Projections (q/k f-tiles for the next pair, V chunks) are interleaved into
  the head loops as PE filler, one item per key-block slot, ordered so no
  PE instruction waits on a DMA that hasn't been issued long before.
- DMAs are emitted in need-order (x strips, pair-0 weights, wv halves, ...,
  wo last) because they are serviced in order.
- Tail: phase-4 out-projection partials for it=0 run as head-10 filler in
  held PSUM accumulators; the last head's normalize runs as two parallel
  half-width chains (Pool+DVE) to cut the latency gating phase 4.
- PSUM: ST double-buffer (4 banks) + proj chunks (2) + O accumulator (2);
  tail reuses the freed ST pool for full-width phase-4 accumulation.

Per-core dataflow (all matmuls float32r, full PE rate at free>=256):
  x [n,c] --PE transpose--> xT [c,n]
  qT,kT per head pair = (w_qkv f-tile).T @ xT   in [d, n] layout
  V = xT.T @ w_qkv[:, 1536:] in [n, d] layout, + ones column (denom fold)
  per head h, key-block jt: ST[k,i] = kT.T @ qT ; E = exp(SCALE*ST) (ACT)
     O'[65, i] += [V|1].T @ E   (row 64 = softmax denominators)
  attnT[d, i] = O'[0:64] * (1/O'[64])
  out[i, e] = attnT.T @ w_out + b_out
"""

import numpy as np
from contextlib import ExitStack

import concourse.bacc as bacc
import concourse.mybir as mybir
import concourse.tile as tile
from concourse.bass import ds, ts
from concourse.bass_utils import run_bass_kernel_spmd
from concourse.masks import make_identity

P = 128
N_CORES = 8
N_TOK = 1024
DIM = 768
H = 12
HD = 64
SCALE = 1.0 / (DIM ** 0.5)
F32 = mybir.dt.float32
F32R = mybir.dt.float32r
EXP = mybir.ActivationFunctionType.Exp

C_T = DIM // P          # 6  c-tiles
N_T = N_TOK // P        # 8  token tiles
HALF = 512


def _emit_body(nc, tc, ctx, pools, dram, skip=()):
    x_d, wqkv_d, wout_d, bout_d, out_d = dram
    (const, persist, qkpool, xpool, wpool, expool, spool, outpool,
     psST, psP, psO) = pools

    wqkv_r = wqkv_d.rearrange("(o p) f -> p o f", p=P)
    wout_r = wout_d.rearrange("(o p) f -> p o f", p=P)

    # ---- constants ----
    identity = const.tile([P, P], F32, tag="ident")
    make_identity(nc, identity[:])
    b_row = const.tile([1, DIM], F32R, tag="brow")
    ones_row = const.tile([1, P], F32R, tag="ones1")
    nc.vector.memset(ones_row[:].bitcast(F32), 1.0)

    # ---- persistent tensors ----
    xT = persist.tile([P, C_T, N_TOK], F32R, tag="xT")
    vplus = persist.tile([P, N_T, H, HD + 1], F32R, tag="vplus")
    attnT = persist.tile([P, C_T, N_TOK], F32R, tag="attnT")
    wv_sb = persist.tile([P, C_T, DIM], F32R, tag="wv")
    wo_sb = persist.tile([P, C_T, DIM], F32R, tag="wo")

    nc.vector.memset(vplus[:, :, :, ds(HD, 1)].bitcast(F32), 1.0)

    # ---- DMA + projection helpers ----
    wt_tiles = {}

    def fetch_wt(fi):
        wt = wpool.tile([P, C_T, P], F32R, tag="wqk", name=f"wt_{fi}")
        nc.sync.dma_start(wt[:], wqkv_r[:, :, ds(fi * P, P)].bitcast(F32R))
        wt_tiles[fi] = wt

    qk_tiles = {}

    def alloc_qk(pair):
        qk_tiles[pair] = qkpool.tile([P, 2, N_TOK], F32R, tag="qkT",
                                     name=f"qkT_{pair}")

    def qk_chunk(pair, qk, chunk):
        """One 512-col chunk of a q or k f-tile projection -> qkT."""
        fi = pair + 6 * qk
        wt = wt_tiles[fi]
        ps = psP.tile([P, HALF], F32, tag="pp", name=f"qk_{fi}_{chunk}")
        for k in range(C_T):
            nc.tensor.matmul(ps[:], wt[:, k], xT[:, k, ds(chunk * HALF, HALF)],
                             start=(k == 0), stop=(k == C_T - 1))
        nc.vector.tensor_copy(qk_tiles[pair][:, qk, ds(chunk * HALF, HALF)], ps[:])

    def v_chunk(jt, chunk):
        """One chunk of the V projection for token-block jt -> vplus."""
        w = HALF if chunk == 0 else DIM - HALF
        ps = psP.tile([P, HALF], F32, tag="pp", name=f"v_{jt}_{chunk}")
        for k in range(C_T):
            nc.tensor.matmul(ps[:, ds(0, w)], xT[:, k, ts(jt, P)],
                             wv_sb[:, k, ds(chunk * HALF, w)],
                             start=(k == 0), stop=(k == C_T - 1))
        h0, nh = (0, 8) if chunk == 0 else (8, 4)
        nc.vector.tensor_copy(
            vplus[:, jt, ds(h0, nh), ds(0, HD)],
            ps[:, ds(0, w)].rearrange("p (h d) -> p h d", d=HD))

    # ---- phase 1: DMAs in need-order + PE transposes ----
    xs_tiles = {}

    def load_x(it):
        xs = xpool.tile([P, DIM], F32, tag="xs", name=f"xs_{it}")
        nc.sync.dma_start(xs[:], x_d[ts(it, P), :])
        xs_tiles[it] = xs

    def transpose_strip(it):
        xs = xs_tiles[it]
        for c in range(0, C_T, 2):
            pst = psST.tile([P, N_TOK], F32, tag="st", name=f"tp_{it}_{c}")
            nc.tensor.transpose(pst[:, 0:P], xs[:, ts(c, P)], identity[:])
            nc.tensor.transpose(pst[:, ds(HALF, P)], xs[:, ts(c + 1, P)],
                                identity[:])
            if (it + c // 2) % 2 == 0:
                nc.vector.tensor_copy(xT[:, c, ts(it, P)], pst[:, 0:P])
                nc.scalar.activation(xT[:, c + 1, ts(it, P)],
                                     pst[:, ds(HALF, P)], mybir.ActivationFunctionType.Copy)
            else:
                nc.scalar.activation(xT[:, c, ts(it, P)], pst[:, 0:P], mybir.ActivationFunctionType.Copy)
                nc.vector.tensor_copy(xT[:, c + 1, ts(it, P)], pst[:, ds(HALF, P)])

    for it in range(4):
        load_x(it)
    fetch_wt(6)            # pair-0 k
    fetch_wt(0)            # pair-0 q
    for it in range(4):
        transpose_strip(it)
    for it in range(4, N_T):
        load_x(it)
    fetch_wt(7)            # pair-1 k
    fetch_wt(1)            # pair-1 q
    nc.sync.dma_start(b_row[:], bout_d[None, :].bitcast(F32R))
    nc.sync.dma_start(wv_sb[:, :, ds(0, HALF)],
                      wqkv_r[:, :, ds(2 * DIM, HALF)].bitcast(F32R))
    alloc_qk(0)
    qk_chunk(0, 1, 0)      # k chunk 0 (needs xT cols 0-511 = strips 0-3)
    for it in range(4, N_T):
        transpose_strip(it)
    nc.sync.dma_start(wv_sb[:, :, ds(HALF, DIM - HALF)],
                      wqkv_r[:, :, ds(2 * DIM + HALF, DIM - HALF)].bitcast(F32R))
    qk_chunk(0, 1, 1)
    qk_chunk(0, 0, 0)
    qk_chunk(0, 0, 1)
    nc.sync.dma_start(wo_sb[:], wout_r[:].bitcast(F32R))

    # ---- per-head filler queues ----
    def head_fillers(h):
        fl = []
        if h == 0:
            alloc_qk(1)
            fl.append(lambda: qk_chunk(1, 1, 0))
            fl.append(lambda: qk_chunk(1, 1, 1))
            for jt in range(N_T):
                fl.append(lambda jt=jt: v_chunk(jt, 0))
        elif h == 1:
            fl.append(lambda: qk_chunk(1, 0, 0))
            fl.append(lambda: qk_chunk(1, 0, 1))
            for jt in range(N_T):
                fl.append(lambda jt=jt: v_chunk(jt, 1))
        elif h < 2 * (C_T - 1):
            npair = h // 2 + 1
            qk = 1 - (h % 2)      # even head -> next pair's k, odd -> q
            fi = npair + 6 * qk

            def start(npair=npair, qk=qk, fi=fi, h=h):
                fetch_wt(fi)
                if h % 2 == 0:
                    alloc_qk(npair)
                qk_chunk(npair, qk, 0)
            fl.append(start)
            fl.append(lambda npair=npair, qk=qk: qk_chunk(npair, qk, 1))
        elif h == H - 2:
            # phase-4 partials for it=0 in held psP accumulators
            psa0 = psP.tile([P, HALF], F32, tag="pp", name="o4a_0")
            psb0 = psP.tile([P, HALF], F32, tag="pp", name="o4b_0")
            tail_state["psa0"], tail_state["psb0"] = psa0, psb0
            for k in range(C_T - 1):
                def p4part(k=k, psa0=psa0, psb0=psb0):
                    nc.tensor.matmul(psa0[:], attnT[:, k, ts(0, P)],
                                     wo_sb[:, k, ds(0, HALF)],
                                     start=(k == 0), stop=False)
                    nc.tensor.matmul(psb0[:, ds(0, DIM - HALF)],
                                     attnT[:, k, ts(0, P)],
                                     wo_sb[:, k, ds(HALF, DIM - HALF)],
                                     start=(k == 0), stop=False)
                fl.append(p4part)
        return fl

    tail_state = {}

    # ---- phase 3: pipelined attention, head-at-a-time ----
    for h in range(H):
        pair, sub = h // 2, h % 2
        qkt = qk_tiles[pair]
        b0 = HD * sub
        lag = 4 if h == 0 else 1
        fl = head_fillers(h)
        o_ps = psO.tile([HD + 1, N_TOK], F32, tag="po", name=f"po_{h}")
        es_tiles = {}

        def emit_av(jt, first, last):
            for c2 in range(2):
                nc.tensor.matmul(
                    o_ps[:, ds(c2 * HALF, HALF)], vplus[:, jt, h],
                    es_tiles[jt][:, ds(c2 * HALF, HALF)],
                    start=first, stop=last)

        for jt in range(N_T):
            st = psST.tile([P, N_TOK], F32, tag="st", name=f"st_{h}_{jt}")
            for c2 in range(2):
                nc.tensor.matmul(
                    st[:, ds(c2 * HALF, HALF)],
                    qkt[ds(b0, HD), 1, ts(jt, P)],
                    qkt[ds(b0, HD), 0, ds(c2 * HALF, HALF)],
                    start=True, stop=True, tile_position=(b0, 0))
            es = expool.tile([P, N_TOK], F32R, tag="es", name=f"es_{h}_{jt}")
            nc.scalar.activation(es[:], st[:], EXP, scale=SCALE)
            es_tiles[jt] = es
            if fl:
                fl.pop(0)()
            if jt - lag >= 0:
                emit_av(jt - lag, jt - lag == 0, False)
        # drain: alternate remaining fillers with remaining attnVs
        pend = list(range(max(0, N_T - lag), N_T))
        while fl or pend:
            if fl:
                fl.pop(0)()
            if pend:
                jt = pend.pop(0)
                emit_av(jt, jt == 0, jt == N_T - 1)

        # normalize (off the PE path)
        if h < H - 1:
            ostg = spool.tile([HD + 1, N_TOK], F32, tag="ostg", name=f"ostg_{h}")
            nc.vector.tensor_copy(ostg[:], o_ps[:HD + 1, :])
            rec = spool.tile([1, N_TOK], F32, tag="rec")
            nc.vector.reciprocal(rec[:], ostg[ds(HD, 1), :])
            rb = spool.tile([HD, N_TOK], F32, tag="rb")
            nc.gpsimd.partition_broadcast(rb[:], rec[:])
            nc.vector.tensor_mul(attnT[ds(b0, HD), pair, :], ostg[0:HD, :], rb[:])
        else:
            # last head: two parallel half-width chains to cut the latency
            # that gates phase 4's final c-tile
            ostg = spool.tile([HD + 1, N_TOK], F32, tag="ostg", name=f"ostg_{h}")
            nc.vector.tensor_copy(ostg[:, ds(0, HALF)], o_ps[:HD + 1, ds(0, HALF)])
            nc.vector.tensor_copy(ostg[:, ds(HALF, HALF)],
                                  o_ps[:HD + 1, ds(HALF, HALF)])
            rec = spool.tile([1, N_TOK], F32, tag="rec")
            nc.vector.reciprocal(rec[:, ds(0, HALF)], ostg[ds(HD, 1), ds(0, HALF)])
            nc.vector.reciprocal(rec[:, ds(HALF, HALF)],
                                 ostg[ds(HD, 1), ds(HALF, HALF)])
            rb = spool.tile([HD, N_TOK], F32, tag="rb")
            nc.gpsimd.partition_broadcast(rb[:, ds(0, HALF)], rec[:, ds(0, HALF)])
            nc.gpsimd.partition_broadcast(rb[:, ds(HALF, HALF)],
                                          rec[:, ds(HALF, HALF)])
            nc.vector.tensor_mul(attnT[ds(b0, HD), pair, ds(0, HALF)],
                                 ostg[0:HD, ds(0, HALF)], rb[:, ds(0, HALF)])
            nc.gpsimd.tensor_mul(attnT[ds(b0, HD), pair, ds(HALF, HALF)],
                                 ostg[0:HD, ds(HALF, HALF)], rb[:, ds(HALF, HALF)])

    # ---- phase 4: output projection + bias ----
    # it=0: finish the held partial accumulators (k=5 only)
    psa0, psb0 = tail_state["psa0"], tail_state["psb0"]
    k = C_T - 1
    nc.tensor.matmul(psa0[:], attnT[:, k, ts(0, P)], wo_sb[:, k, ds(0, HALF)],
                     start=False, stop=False)
    nc.tensor.matmul(psb0[:, ds(0, DIM - HALF)], attnT[:, k, ts(0, P)],
                     wo_sb[:, k, ds(HALF, DIM - HALF)], start=False, stop=False)
    nc.tensor.matmul(psa0[:], ones_row[:], b_row[:, ds(0, HALF)],
                     start=False, stop=True)
    nc.tensor.matmul(psb0[:, ds(0, DIM - HALF)], ones_row[:],
                     b_row[:, ds(HALF, DIM - HALF)], start=False, stop=True)
    os0 = outpool.tile([P, DIM], F32, tag="os", name="os_0")
    nc.scalar.activation(os0[:, ds(0, HALF)], psa0[:], mybir.ActivationFunctionType.Copy)
    nc.scalar.activation(os0[:, ds(HALF, DIM - HALF)], psb0[:, ds(0, DIM - HALF)],
                         mybir.ActivationFunctionType.Copy)
    nc.sync.dma_start(out_d[ts(0, P), :], os0[:])

    # it=1..7: full-width accumulation in the freed ST pool (2 banks/tile)
    for it in range(1, N_T):
        ps = psST.tile([P, N_TOK], F32, tag="st", name=f"o4_{it}")
        for k in range(C_T):
            nc.tensor.matmul(ps[:, ds(0, HALF)], attnT[:, k, ts(it, P)],
                             wo_sb[:, k, ds(0, HALF)],
                             start=(k == 0), stop=False)
        nc.tensor.matmul(ps[:, ds(0, HALF)], ones_row[:], b_row[:, ds(0, HALF)],
                         start=False, stop=True)
        for k in range(C_T):
            nc.tensor.matmul(ps[:, ds(HALF, DIM - HALF)], attnT[:, k, ts(it, P)],
                             wo_sb[:, k, ds(HALF, DIM - HALF)],
                             start=(k == 0), stop=False)
        nc.tensor.matmul(ps[:, ds(HALF, DIM - HALF)], ones_row[:],
                         b_row[:, ds(HALF, DIM - HALF)], start=False, stop=True)
        os = outpool.tile([P, DIM], F32, tag="os", name=f"os_{it}")
        if it % 2 == 0:
            nc.scalar.activation(os[:], ps[:, ds(0, DIM)], mybir.ActivationFunctionType.Copy)
        else:
            nc.vector.tensor_copy(os[:], ps[:, ds(0, DIM)])
        nc.sync.dma_start(out_d[ts(it, P), :], os[:])


def build_nc(reps: int = 1, timing_mode: bool = False, skip=()):
    nc = bacc.Bacc("TRN2", target_bir_lowering=False, debug=False)
    if timing_mode:
        # device-resident garbage inputs: measure kernel exec, not host I/O
        x_d = nc.dram_tensor("x", [N_TOK, DIM], F32).ap()
        wqkv_d = nc.dram_tensor("w_qkv", [DIM, 3 * DIM], F32).ap()
        wout_d = nc.dram_tensor("w_out", [DIM, DIM], F32).ap()
        bout_d = nc.dram_tensor("b_out", [DIM], F32).ap()
        out_d = nc.dram_tensor("out", [N_TOK, DIM], F32).ap()
        dummy_in = nc.dram_tensor("dummy_in", [1, 1], F32, kind="ExternalInput").ap()
        tiny_out = nc.dram_tensor("tiny_out", [1, 1], F32, kind="ExternalOutput").ap()
    else:
        x_d = nc.dram_tensor("x", [N_TOK, DIM], F32, kind="ExternalInput").ap()
        wqkv_d = nc.dram_tensor("w_qkv", [DIM, 3 * DIM], F32, kind="ExternalInput").ap()
        wout_d = nc.dram_tensor("w_out", [DIM, DIM], F32, kind="ExternalInput").ap()
        bout_d = nc.dram_tensor("b_out", [DIM], F32, kind="ExternalInput").ap()
        out_d = nc.dram_tensor("out", [N_TOK, DIM], F32, kind="ExternalOutput").ap()
    dram = (x_d, wqkv_d, wout_d, bout_d, out_d)

    with ExitStack() as ctx:
        tc = ctx.enter_context(tile.TileContext(nc))
        const = ctx.enter_context(tc.tile_pool(name="const", bufs=1))
        persist = ctx.enter_context(tc.tile_pool(name="persist", bufs=1))
        qkpool = ctx.enter_context(tc.tile_pool(name="qkpool", bufs=2))
        xpool = ctx.enter_context(tc.tile_pool(name="xpool", bufs=4))
        wpool = ctx.enter_context(tc.tile_pool(name="wpool", bufs=3))
        expool = ctx.enter_context(tc.tile_pool(name="expool", bufs=6))
        spool = ctx.enter_context(tc.tile_pool(name="spool", bufs=2))
        outpool = ctx.enter_context(tc.tile_pool(name="outpool", bufs=2))
        psST = ctx.enter_context(tc.tile_pool(name="psST", bufs=2, space="PSUM"))
        psP = ctx.enter_context(tc.tile_pool(name="psP", bufs=2, space="PSUM"))
        psO = ctx.enter_context(tc.tile_pool(name="psO", bufs=1, space="PSUM"))
        pools = (const, persist, qkpool, xpool, wpool, expool, spool, outpool,
                 psST, psP, psO)

        if reps == 1:
            _emit_body(nc, tc, ctx, pools, dram, skip=skip)
        else:
            with tc.For_i(0, reps, 1):
                _emit_body(nc, tc, ctx, pools, dram, skip=skip)
        if timing_mode:
            tz = const.tile([1, 1], F32, tag="tz")
            nc.sync.dma_start(tz[:], dummy_in[:])
            nc.sync.dma_start(tiny_out[:], tz[:])

    nc.compile()
    return nc


_NC_CACHE = {}


def kernel(**inputs) -> np.ndarray:
    x = np.ascontiguousarray(np.asarray(inputs["x"], dtype=np.float32))
    w_qkv = np.ascontiguousarray(np.asarray(inputs["w_qkv"], dtype=np.float32))
    w_out = np.ascontiguousarray(np.asarray(inputs["w_out"], dtype=np.float32))
    b_out = np.ascontiguousarray(np.asarray(inputs["b_out"], dtype=np.float32))

    if "nc" not in _NC_CACHE:
        _NC_CACHE["nc"] = build_nc(reps=1)
    nc = _NC_CACHE["nc"]

    in_maps = [
        {"x": x[c], "w_qkv": w_qkv, "w_out": w_out, "b_out": b_out}
        for c in range(N_CORES)
    ]
    res = run_bass_kernel_spmd(nc, in_maps, core_ids=list(range(N_CORES)))
    out = np.stack([res.results[c]["out"] for c in range(N_CORES)], axis=0)
    return out.astype(np.float32)


# revision 10
# speedup vs baseline: 1.6577x; 1.6577x over previous
"""Trainium2 Bass kernel for nn_Attention (b=8, n=1024, dim=768, heads=12).

Sharding: data-parallel over batch — 8 batch elements -> 8 NeuronCores.
Each core runs full attention for one [1024, 768] slice; weights replicated.

v2b: software-pipelined schedule.
- Head-at-a-time attention; attnV lags the ST/exp pipeline (lag 4 for head 0
  to ride out the wv DMA arrival, lag 1 after) so the exp (ACT engine)
  overlaps PE work instead of stalling it.
- Projections (q/k f-tiles for the next pair, V chunks) are interleaved into
  the head loops as PE filler, one item per key-block slot, ordered so no
  PE instruction waits on a DMA that hasn't been issued long before.
- DMAs are emitted in need-order (x strips, pair-0 weights, wv halves, ...,
  wo last) because they are serviced in order.
- Tail: phase-4 out-projection partials for it=0 run as head-10 filler in
  held PSUM accumulators; the last head's normalize runs as two parallel
  half-width chains (Pool+DVE) to cut the latency gating phase 4.
- PSUM: ST double-buffer (4 banks) + proj chunks (2) + O accumulator (2);
  tail reuses the freed ST pool for full-width phase-4 accumulation.

Per-core dataflow (all matmuls float32r, full PE rate at free>=256):
  x [n,c] --PE transpose--> xT [c,n]
  qT,kT per head pair = (w_qkv f-tile).T @ xT   in [d, n] layout
  V = xT.T @ w_qkv[:, 1536:] in [n, d] layout, + ones column (denom fold)
  per head h, key-block jt: ST[k,i] = kT.T @ qT ; E = exp(SCALE*ST) (ACT)
     O'[65, i] += [V|1].T @ E   (row 64 = softmax denominators)
  attnT[d, i] = O'[0:64] * (1/O'[64])
  out[i, e] = attnT.T @ w_out + b_out
"""

import numpy as np
from contextlib import ExitStack

import concourse.bacc as bacc
import concourse.mybir as mybir
import concourse.tile as tile
from concourse.bass import ds, ts
from concourse.bass_utils import run_bass_kernel_spmd
from concourse.masks import make_identity

P = 128
N_CORES = 8
N_TOK = 1024
DIM = 768
H = 12
HD = 64
SCALE = 1.0 / (DIM ** 0.5)
F32 = mybir.dt.float32
F32R = mybir.dt.float32r
EXP = mybir.ActivationFunctionType.Exp

C_T = DIM // P          # 6  c-tiles
N_T = N_TOK // P        # 8  token tiles
HALF = 512


def _emit_body(nc, tc, ctx, pools, dram, skip=()):
    x_d, wqkv_d, wout_d, bout_d, out_d = dram
    (const, persist, qkpool, xpool, wpool, expool, spool, outpool,
     psST, psP, psO) = pools

    wqkv_r = wqkv_d.rearrange("(o p) f -> p o f", p=P)
    wout_r = wout_d.rearrange("(o p) f -> p o f", p=P)

    # ---- constants ----
    identity = const.tile([P, P], F32, tag="ident")
    make_identity(nc, identity[:])
    b_row = const.tile([1, DIM], F32R, tag="brow")
    bias_bc = const.tile([P, DIM], F32, tag="bias")

    # ---- persistent tensors ----
    xT = persist.tile([P, C_T, N_TOK], F32R, tag="xT")
    vplus = persist.tile([P, N_T, H, HD + 1], F32R, tag="vplus")
    attnT = persist.tile([P, C_T, N_TOK], F32R, tag="attnT")
    wv_sb = persist.tile([P, C_T, DIM], F32R, tag="wv")
    wo_sb = persist.tile([P, C_T, DIM], F32R, tag="wo")

    nc.vector.memset(vplus[:, :, :, ds(HD, 1)].bitcast(F32), 1.0)

    # ---- DMA + projection helpers ----
    wt_tiles = {}

    def fetch_wt(fi):
        wt = wpool.tile([P, C_T, P], F32R, tag="wqk", name=f"wt_{fi}")
        nc.sync.dma_start(wt[:], wqkv_r[:, :, ds(fi * P, P)].bitcast(F32R))
        wt_tiles[fi] = wt

    qk_tiles = {}

    def alloc_qk(pair):
        qk_tiles[pair] = qkpool.tile([P, 2, N_TOK], F32R, tag="qkT",
                                     name=f"qkT_{pair}")

    def qk_chunk(pair, qk, chunk):
        """One 512-col chunk of a q or k f-tile projection -> qkT."""
        fi = pair + 6 * qk
        wt = wt_tiles[fi]
        ps = psP.tile([P, HALF], F32, tag="pp", name=f"qk_{fi}_{chunk}")
        for k in range(C_T):
            nc.tensor.matmul(ps[:], wt[:, k], xT[:, k, ds(chunk * HALF, HALF)],
                             start=(k == 0), stop=(k == C_T - 1))
        nc.vector.tensor_copy(qk_tiles[pair][:, qk, ds(chunk * HALF, HALF)], ps[:])

    def v_chunk(jt, chunk):
        """One chunk of the V projection for token-block jt -> vplus."""
        w = HALF if chunk == 0 else DIM - HALF
        ps = psP.tile([P, HALF], F32, tag="pp", name=f"v_{jt}_{chunk}")
        for k in range(C_T):
            nc.tensor.matmul(ps[:, ds(0, w)], xT[:, k, ts(jt, P)],
                             wv_sb[:, k, ds(chunk * HALF, w)],
                             start=(k == 0), stop=(k == C_T - 1))
        h0, nh = (0, 8) if chunk == 0 else (8, 4)
        nc.vector.tensor_copy(
            vplus[:, jt, ds(h0, nh), ds(0, HD)],
            ps[:, ds(0, w)].rearrange("p (h d) -> p h d", d=HD))

    # ---- phase 1: DMAs in need-order + PE transposes ----
    xs_tiles = {}

    def load_x(it):
        xs = xpool.tile([P, DIM], F32, tag="xs", name=f"xs_{it}")
        nc.sync.dma_start(xs[:], x_d[ts(it, P), :])
        xs_tiles[it] = xs

    def transpose_strip(it):
        xs = xs_tiles[it]
        for c in range(0, C_T, 2):
            pst = psST.tile([P, N_TOK], F32, tag="st", name=f"tp_{it}_{c}")
            nc.tensor.transpose(pst[:, 0:P], xs[:, ts(c, P)], identity[:])
            nc.tensor.transpose(pst[:, ds(HALF, P)], xs[:, ts(c + 1, P)],
                                identity[:])
            nc.vector.tensor_copy(xT[:, c, ts(it, P)], pst[:, 0:P])
            nc.vector.tensor_copy(xT[:, c + 1, ts(it, P)], pst[:, ds(HALF, P)])

    for it in range(4):
        load_x(it)
    fetch_wt(6)            # pair-0 k
    fetch_wt(0)            # pair-0 q
    for it in range(4):
        transpose_strip(it)
    for it in range(4, N_T):
        load_x(it)
    fetch_wt(7)            # pair-1 k
    fetch_wt(1)            # pair-1 q
    nc.sync.dma_start(b_row[:], bout_d[None, :].bitcast(F32R))
    nc.gpsimd.partition_broadcast(bias_bc[:], b_row[:].bitcast(F32))
    nc.sync.dma_start(wv_sb[:, :, ds(0, HALF)],
                      wqkv_r[:, :, ds(2 * DIM, HALF)].bitcast(F32R))
    alloc_qk(0)
    qk_chunk(0, 1, 0)      # k chunk 0 (needs xT cols 0-511 = strips 0-3)
    for it in range(4, N_T):
        transpose_strip(it)
    nc.sync.dma_start(wv_sb[:, :, ds(HALF, DIM - HALF)],
                      wqkv_r[:, :, ds(2 * DIM + HALF, DIM - HALF)].bitcast(F32R))
    qk_chunk(0, 1, 1)
    qk_chunk(0, 0, 0)
    qk_chunk(0, 0, 1)
    nc.sync.dma_start(wo_sb[:], wout_r[:].bitcast(F32R))

    # ---- per-head filler queues ----
    def head_fillers(h):
        fl = []
        if h == 0:
            alloc_qk(1)
            fl.append(lambda: qk_chunk(1, 1, 0))
            fl.append(lambda: qk_chunk(1, 1, 1))
            for jt in range(N_T):
                fl.append(lambda jt=jt: v_chunk(jt, 0))
        elif h == 1:
            fl.append(lambda: qk_chunk(1, 0, 0))
            fl.append(lambda: qk_chunk(1, 0, 1))
            for jt in range(N_T):
                fl.append(lambda jt=jt: v_chunk(jt, 1))
        elif h < 2 * (C_T - 1):
            npair = h // 2 + 1
            qk = 1 - (h % 2)      # even head -> next pair's k, odd -> q
            fi = npair + 6 * qk

            def start(npair=npair, qk=qk, fi=fi, h=h):
                fetch_wt(fi)
                if h % 2 == 0:
                    alloc_qk(npair)
                qk_chunk(npair, qk, 0)
            fl.append(start)
            fl.append(lambda npair=npair, qk=qk: qk_chunk(npair, qk, 1))
        elif h == H - 2:
            # phase-4 partials for it=0 in held psP accumulators
            psa0 = psP.tile([P, HALF], F32, tag="pp", name="o4a_0")
            psb0 = psP.tile([P, HALF], F32, tag="pp", name="o4b_0")
            tail_state["psa0"], tail_state["psb0"] = psa0, psb0
            for k in range(C_T - 1):
                def p4part(k=k, psa0=psa0, psb0=psb0):
                    nc.tensor.matmul(psa0[:], attnT[:, k, ts(0, P)],
                                     wo_sb[:, k, ds(0, HALF)],
                                     start=(k == 0), stop=False)
                    nc.tensor.matmul(psb0[:, ds(0, DIM - HALF)],
                                     attnT[:, k, ts(0, P)],
                                     wo_sb[:, k, ds(HALF, DIM - HALF)],
                                     start=(k == 0), stop=False)
                fl.append(p4part)
        return fl

    tail_state = {}

    # ---- phase 3: pipelined attention, head-at-a-time ----
    for h in range(H):
        pair, sub = h // 2, h % 2
        qkt = qk_tiles[pair]
        b0 = HD * sub
        lag = 4 if h == 0 else 1
        fl = head_fillers(h)
        o_ps = psO.tile([HD + 1, N_TOK], F32, tag="po", name=f"po_{h}")
        es_tiles = {}

        def emit_av(jt, first, last):
            for c2 in range(2):
                nc.tensor.matmul(
                    o_ps[:, ds(c2 * HALF, HALF)], vplus[:, jt, h],
                    es_tiles[jt][:, ds(c2 * HALF, HALF)],
                    start=first, stop=last)

        for jt in range(N_T):
            st = psST.tile([P, N_TOK], F32, tag="st", name=f"st_{h}_{jt}")
            for c2 in range(2):
                nc.tensor.matmul(
                    st[:, ds(c2 * HALF, HALF)],
                    qkt[ds(b0, HD), 1, ts(jt, P)],
                    qkt[ds(b0, HD), 0, ds(c2 * HALF, HALF)],
                    start=True, stop=True, tile_position=(b0, 0))
            es = expool.tile([P, N_TOK], F32R, tag="es", name=f"es_{h}_{jt}")
            nc.scalar.activation(es[:], st[:], EXP, scale=SCALE)
            es_tiles[jt] = es
            if fl:
                fl.pop(0)()
            if jt - lag >= 0:
                emit_av(jt - lag, jt - lag == 0, False)
        # drain: alternate remaining fillers with remaining attnVs
        pend = list(range(max(0, N_T - lag), N_T))
        while fl or pend:
            if fl:
                fl.pop(0)()
            if pend:
                jt = pend.pop(0)
                emit_av(jt, jt == 0, jt == N_T - 1)

        # normalize (off the PE path)
        if h < H - 1:
            ostg = spool.tile([HD + 1, N_TOK], F32, tag="ostg", name=f"ostg_{h}")
            nc.vector.tensor_copy(ostg[:], o_ps[:HD + 1, :])
            rec = spool.tile([1, N_TOK], F32, tag="rec")
            nc.vector.reciprocal(rec[:], ostg[ds(HD, 1), :])
            rb = spool.tile([HD, N_TOK], F32, tag="rb")
            nc.gpsimd.partition_broadcast(rb[:], rec[:])
            nc.vector.tensor_mul(attnT[ds(b0, HD), pair, :], ostg[0:HD, :], rb[:])
        else:
            # last head: two parallel half-width chains to cut the latency
            # that gates phase 4's final c-tile
            ostg = spool.tile([HD + 1, N_TOK], F32, tag="ostg", name=f"ostg_{h}")
            nc.vector.tensor_copy(ostg[:, ds(0, HALF)], o_ps[:HD + 1, ds(0, HALF)])
            nc.vector.tensor_copy(ostg[:, ds(HALF, HALF)],
                                  o_ps[:HD + 1, ds(HALF, HALF)])
            rec = spool.tile([1, N_TOK], F32, tag="rec")
            nc.vector.reciprocal(rec[:, ds(0, HALF)], ostg[ds(HD, 1), ds(0, HALF)])
            nc.vector.reciprocal(rec[:, ds(HALF, HALF)],
                                 ostg[ds(HD, 1), ds(HALF, HALF)])
            rb = spool.tile([HD, N_TOK], F32, tag="rb")
            nc.gpsimd.partition_broadcast(rb[:, ds(0, HALF)], rec[:, ds(0, HALF)])
            nc.gpsimd.partition_broadcast(rb[:, ds(HALF, HALF)],
                                          rec[:, ds(HALF, HALF)])
            nc.vector.tensor_mul(attnT[ds(b0, HD), pair, ds(0, HALF)],
                                 ostg[0:HD, ds(0, HALF)], rb[:, ds(0, HALF)])
            nc.gpsimd.tensor_mul(attnT[ds(b0, HD), pair, ds(HALF, HALF)],
                                 ostg[0:HD, ds(HALF, HALF)], rb[:, ds(HALF, HALF)])

    # ---- phase 4: output projection + bias ----
    # it=0: finish the held partial accumulators (k=5 only)
    psa0, psb0 = tail_state["psa0"], tail_state["psb0"]
    k = C_T - 1
    nc.tensor.matmul(psa0[:], attnT[:, k, ts(0, P)], wo_sb[:, k, ds(0, HALF)],
                     start=False, stop=True)
    nc.tensor.matmul(psb0[:, ds(0, DIM - HALF)], attnT[:, k, ts(0, P)],
                     wo_sb[:, k, ds(HALF, DIM - HALF)], start=False, stop=True)
    os0 = outpool.tile([P, DIM], F32, tag="os", name="os_0")
    nc.vector.tensor_add(os0[:, ds(0, HALF)], psa0[:], bias_bc[:, ds(0, HALF)])
    nc.vector.tensor_add(os0[:, ds(HALF, DIM - HALF)],
                         psb0[:, ds(0, DIM - HALF)],
                         bias_bc[:, ds(HALF, DIM - HALF)])
    nc.sync.dma_start(out_d[ts(0, P), :], os0[:])

    # it=1..7: full-width accumulation in the freed ST pool (2 banks/tile)
    for it in range(1, N_T):
        ps = psST.tile([P, N_TOK], F32, tag="st", name=f"o4_{it}")
        for k in range(C_T):
            nc.tensor.matmul(ps[:, ds(0, HALF)], attnT[:, k, ts(it, P)],
                             wo_sb[:, k, ds(0, HALF)],
                             start=(k == 0), stop=(k == C_T - 1))
        for k in range(C_T):
            nc.tensor.matmul(ps[:, ds(HALF, DIM - HALF)], attnT[:, k, ts(it, P)],
                             wo_sb[:, k, ds(HALF, DIM - HALF)],
                             start=(k == 0), stop=(k == C_T - 1))
        os = outpool.tile([P, DIM], F32, tag="os", name=f"os_{it}")
        nc.vector.tensor_add(os[:, ds(0, HALF)], ps[:, ds(0, HALF)],
                             bias_bc[:, ds(0, HALF)])
        nc.vector.tensor_add(os[:, ds(HALF, DIM - HALF)],
                             ps[:, ds(HALF, DIM - HALF)],
                             bias_bc[:, ds(HALF, DIM - HALF)])
        nc.sync.dma_start(out_d[ts(it, P), :], os[:])


def build_nc(reps: int = 1, timing_mode: bool = False, skip=()):
    nc = bacc.Bacc("TRN2", target_bir_lowering=False, debug=False)
    if timing_mode:
        # device-resident garbage inputs: measure kernel exec, not host I/O
        x_d = nc.dram_tensor("x", [N_TOK, DIM], F32).ap()
        wqkv_d = nc.dram_tensor("w_qkv", [DIM, 3 * DIM], F32).ap()
        wout_d = nc.dram_tensor("w_out", [DIM, DIM], F32).ap()
        bout_d = nc.dram_tensor("b_out", [DIM], F32).ap()
        out_d = nc.dram_tensor("out", [N_TOK, DIM], F32).ap()
        dummy_in = nc.dram_tensor("dummy_in", [1, 1], F32, kind="ExternalInput").ap()
        tiny_out = nc.dram_tensor("tiny_out", [1, 1], F32, kind="ExternalOutput").ap()
    else:
        x_d = nc.dram_tensor("x", [N_TOK, DIM], F32, kind="ExternalInput").ap()
        wqkv_d = nc.dram_tensor("w_qkv", [DIM, 3 * DIM], F32, kind="ExternalInput").ap()
        wout_d = nc.dram_tensor("w_out", [DIM, DIM], F32, kind="ExternalInput").ap()
        bout_d = nc.dram_tensor("b_out", [DIM], F32, kind="ExternalInput").ap()
        out_d = nc.dram_tensor("out", [N_TOK, DIM], F32, kind="ExternalOutput").ap()
    dram = (x_d, wqkv_d, wout_d, bout_d, out_d)

    with ExitStack() as ctx:
        tc = ctx.enter_context(tile.TileContext(nc))
        const = ctx.enter_context(tc.tile_pool(name="const", bufs=1))
        persist = ctx.enter_context(tc.tile_pool(name="persist", bufs=1))
        qkpool = ctx.enter_context(tc.tile_pool(name="qkpool", bufs=2))
        xpool = ctx.enter_context(tc.tile_pool(name="xpool", bufs=4))
        wpool = ctx.enter_context(tc.tile_pool(name="wpool", bufs=3))
        expool = ctx.enter_context(tc.tile_pool(name="expool", bufs=6))
        spool = ctx.enter_context(tc.tile_pool(name="spool", bufs=2))
        outpool = ctx.enter_context(tc.tile_pool(name="outpool", bufs=2))
        psST = ctx.enter_context(tc.tile_pool(name="psST", bufs=2, space="PSUM"))
        psP = ctx.enter_context(tc.tile_pool(name="psP", bufs=2, space="PSUM"))
        psO = ctx.enter_context(tc.tile_pool(name="psO", bufs=1, space="PSUM"))
        pools = (const, persist, qkpool, xpool, wpool, expool, spool, outpool,
                 psST, psP, psO)

        if reps == 1:
            _emit_body(nc, tc, ctx, pools, dram, skip=skip)
        else:
            with tc.For_i(0, reps, 1):
                _emit_body(nc, tc, ctx, pools, dram, skip=skip)
        if timing_mode:
            tz = const.tile([1, 1], F32, tag="tz")
            nc.sync.dma_start(tz[:], dummy_in[:])
            nc.sync.dma_start(tiny_out[:], tz[:])

    nc.compile()
    return nc


_NC_CACHE = {}


def kernel(**inputs) -> np.ndarray:
    x = np.ascontiguousarray(np.asarray(inputs["x"], dtype=np.float32))
    w_qkv = np.ascontiguousarray(np.asarray(inputs["w_qkv"], dtype=np.float32))
    w_out = np.ascontiguousarray(np.asarray(inputs["w_out"], dtype=np.float32))
    b_out = np.ascontiguousarray(np.asarray(inputs["b_out"], dtype=np.float32))

    if "nc" not in _NC_CACHE:
        _NC_CACHE["nc"] = build_nc(reps=1)
    nc = _NC_CACHE["nc"]

    in_maps = [
        {"x": x[c], "w_qkv": w_qkv, "w_out": w_out, "b_out": b_out}
        for c in range(N_CORES)
    ]
    res = run_bass_kernel_spmd(nc, in_maps, core_ids=list(range(N_CORES)))
    out = np.stack([res.results[c]["out"] for c in range(N_CORES)], axis=0)
    return out.astype(np.float32)
